# revision 12
# baseline (speedup 1.0000x reference)
"""Trainium2 Bass kernel for ChannelFeatures (channel-attention style module).

Computes, per batch element b:
    x_max[b] = max over (H,W) of features[b]          # (C,)
    x_avg[b] = mean over (H,W) of features[b]         # (C,)
    7 residual blocks (shared weights on both branches):
        x = prelu(W1[k] @ x + b1[k], a1[k]) + x
    scores[b] = sigmoid(x_max[b] + x_avg[b])          # (C,)
    out[b] = features[b] * scores[b]                  # broadcast over (H,W)

Sharding: pure data parallel over batch — 16 batch elements across 8 cores,
2 per core, weights replicated. No cross-core communication.

Device strategy per core (2 batch elements, each (65536, 64) fp32 = 16.8 MB):
  The whole shard is cached in SBUF as fp16 (2 x 8 MB), so features are read
  from HBM exactly once and written exactly once (67.1 MB/core total traffic,
  the memory-roofline minimum). The fp32->fp16 cast happens inside the load
  DMA (SWDGE cast), the fp16->fp32 cast inside the store DMA.

  Layout: partition p holds spatial rows [p*512, (p+1)*512) of a batch
  (one contiguous 128 KB DRAM chunk per partition -> near-optimal DMA
  descriptors). Loads/stores move 4 MB chunks (128 q-rows each).

  Reductions: per-channel max via running elementwise fp16 tensor_max over
  chunks (DVE 16-bit 2x mode) + a log2 fold tree + PE transpose for the
  cross-partition step; per-channel sum via PE ones-matmul over fp16
  segments accumulated in PSUM (mean scale folded into a K=1 matmul).

  Recurrence: bias row folded into an augmented (65, C) weight matmul;
  PReLU+residual fused into 3 DVE scalar_tensor_tensor ops per block.

  Scores are broadcast to 128 partitions with a K=1 ones-matmul (no DRAM
  bounce), then each chunk is scaled in place (fp16 2x tensor_mul) and
  cast-stored back to fp32 DRAM.
"""

import numpy as np
from contextlib import ExitStack

import concourse.bass as bass
import concourse.tile as tile
from concourse import mybir
from concourse.bass_utils import run_bass_kernel_spmd

# Problem shapes (hardcoded per contract)
B, H, W, C = 16, 256, 256, 64
CONV_NUM = 7
NCORES = 8
BPC = B // NCORES          # batch elements per core
HW = H * W                 # 65536 spatial positions
P = 128                    # SBUF partitions
Q = HW // P                # 512 spatial rows per partition per batch
NLD = 2                    # load DMAs per batch (8.4 MB each)
QL = Q // NLD              # 256 q-rows per load chunk
NMX = 4                    # tensor_max granularity: quarter-batch views
CHF = (Q // NMX) * C       # 8192 elems per partition per max-op
QS = 64                    # q-rows per store chunk (2 MB fp32)
NST = Q // QS              # 8 store chunks per batch
FREE = Q * C               # 32768 fp16 elems per partition per batch
SEG = 512                  # PSUM-bank-width matmul segments
NSEG = FREE // SEG         # 64 sum-matmul segments per batch
F32 = mybir.dt.float32
F16 = mybir.dt.float16

AL = mybir.AluOpType

# test.py hooks: set PROFILE=True before calling kernel() to capture an NTFF
# trace; LAST_EXEC_NS then holds the max per-core HW execution time.
PROFILE = False
LAST_EXEC_NS = None
LAST_RESULTS = None


def _split_dma_waits(nc: bass.Bass) -> None:
    """The pinned walrus build rejects DMA instructions carrying more than one
    sync-wait ("Too many sync wait commands"). Tile's sem assignment is not
    transitively minimal, so slot-reuse instructions can get two waits
    (consumer release + WAW with the previous writer). Hoist all but the last
    wait onto wait-only EventSemaphore instructions on the same engine right
    before the instruction."""
    n = 0
    # num=200: outside every id Tile allocated (its end-of-kernel range-clear
    # covers the allocated block), so no collision with released Tile sems.
    dummy = nc.alloc_semaphore(name="wsplit_dummy", num=200)
    for fn in nc.m.functions:
        for blk in fn.blocks:
            new_insts = []
            for inst in blk.instructions:
                si = getattr(inst, "sync_info", None)
                if si is not None and len(si.on_wait) > 1:
                    for w in si.on_wait[:-1]:
                        ev = mybir.InstEventSemaphore(
                            name=f"WSPLIT-{n}", ins=[], outs=[]
                        )
                        n += 1
                        ev.engine = inst.engine
                        # Tick a dedicated dummy sem nobody waits on, so the
                        # simulator/race tooling (which require every
                        # instruction to carry an update) accept the carrier.
                        upd = mybir.SyncUpdate(
                            sync_type="semaphore",
                            id=dummy.num,
                            ant_name=dummy.name,
                            update_mode="sem-add-imm",
                            update_value=1,
                        )
                        ev.sync_info = mybir.SyncInfo(on_wait=[w], on_update=[upd])
                        new_insts.append(ev)
                    si.on_wait = [si.on_wait[-1]]
                new_insts.append(inst)
            blk.instructions = new_insts


def _build_nc() -> bass.Bass:
    nc = bass.Bass()
    feat = nc.declare_dram_parameter("features", [BPC, HW, C], F32, isOutput=False)
    # augmented lhsT: rows 0..C-1 = W1[k].T columns, row C = b1[k] (bias fold)
    waugT = nc.declare_dram_parameter("waugT", [C + 1, CONV_NUM, C], F32, isOutput=False)
    aT = nc.declare_dram_parameter("aT", [C, CONV_NUM], F32, isOutput=False)
    ident = nc.declare_dram_parameter("ident", [P, P], F32, isOutput=False)
    out = nc.declare_dram_parameter("out", [BPC, HW, C], F32, isOutput=True)

    feat_r = feat[:].rearrange("b (p q) c -> b p q c", p=P)
    out_r = out[:].rearrange("b (p q) c -> b p q c", p=P)

    with ExitStack() as ctx:
        tc = ctx.enter_context(tile.TileContext(nc))
        singles = ctx.enter_context(tc.tile_pool(name="singles", bufs=1))
        small = ctx.enter_context(tc.tile_pool(name="small", bufs=3))
        stpool = ctx.enter_context(tc.tile_pool(name="stage", bufs=3))
        psum_sum = ctx.enter_context(tc.tile_pool(name="psum_sum", bufs=1, space="PSUM"))
        psum_m = ctx.enter_context(tc.tile_pool(name="psum_m", bufs=1, space="PSUM"))

        # ---- constants (HWDGE loads; POOL queue stays clear for the stream)
        w_sb = singles.tile([C + 1, CONV_NUM, C], F32)   # [c_in_aug, k, c_out]
        nc.sync.dma_start(out=w_sb[:], in_=waugT[:])
        a_sb = singles.tile([C, CONV_NUM], F32)          # [c, k] (a1[k] per row)
        nc.sync.dma_start(out=a_sb[:], in_=aT[:])
        ones_mm = singles.tile([P, 1], F16)              # sum-matmul lhsT
        nc.vector.memset(ones_mm[:], 1.0)
        ones_row = singles.tile([1, P], F32)             # broadcast-matmul lhsT
        nc.vector.memset(ones_row[:], 1.0)
        one_hw = singles.tile([1, 1], F32)
        nc.vector.memset(one_hw[:], 1.0 / HW)
        identity = singles.tile([P, P], F32)
        nc.sync.dma_start(out=identity[:], in_=ident[:])

        # persistent per-batch state
        cb = [singles.tile([P, Q, C], F16, tag=f"cb{b}", name=f"cb{b}") for b in range(BPC)]
        cbf = [t[:].rearrange("p q c -> p (q c)") for t in cb]
        rmax_t = singles.tile([P, CHF], F16, tag="rmax", name="rmax")
        rmax = [rmax_t for _ in range(BPC)]
        xaug = [singles.tile([C + 1, 2], F32, tag=f"xaug{b}", name=f"xaug{b}") for b in range(BPC)]
        bcb = [singles.tile([P, C], F16, tag=f"bcb{b}", name=f"bcb{b}") for b in range(BPC)]

        for b in range(BPC):
            nc.vector.memset(xaug[b][C : C + 1, :], 1.0)

        # ---- loads: all chunk cast-DMAs up front on the gpsimd (SWDGE) ring
        for b in range(BPC):
            for j in range(NLD):
                nc.gpsimd.dma_start(
                    out=cb[b][:, j * QL : (j + 1) * QL, :],
                    in_=feat_r[b, :, j * QL : (j + 1) * QL, :],
                )

        def emit_tmax(b, j):
            # running elementwise max over quarter-batch views (fp16 2x TT);
            # first quarter initializes via copy (4x mode), no memset needed
            chunk = cbf[b][:, j * CHF : (j + 1) * CHF]
            if j == 0:
                nc.vector.tensor_copy(rmax[b][:], chunk)
            else:
                nc.vector.tensor_max(rmax[b][:], rmax[b][:], chunk)

        # ---- per-batch: reduce -> recurrence -> scores -> scale -> store
        for b in range(BPC):
            if b > 0:
                # scheduler-only fence: keep batch b's section behind batch
                # b-1's score chain in every engine's static order (the
                # scheduler's DMA cost model diverges from reality, which
                # otherwise buries b-1's tiny critical chain behind b's bulk)
                tc.no_sync_barrier()
            else:
                for j in range(NMX):
                    emit_tmax(b, j)
            # per-channel sums on PE: ones^T @ segment, PSUM-accumulated
            ps = psum_sum.tile([1, SEG], F32, tag=f"ps{b}")
            for s in range(NSEG):
                nc.tensor.matmul(
                    ps[:],
                    ones_mm[:],
                    cbf[b][:, s * SEG : (s + 1) * SEG],
                    start=(s == 0),
                    stop=(s == NSEG - 1),
                )
            # The fold->scores->multiply chain gates when stores can start;
            # pull it forward in the static schedule so batch 0's stores
            # overlap batch 1's loads on the DMA ring.
            with tc.high_priority(offset=100000):
                # fold tree: (P, 8192) -> (P, 64), c-alignment preserved
                h = CHF // 2
                while h >= C:
                    nc.vector.tensor_max(
                        rmax[b][:, 0:h], rmax[b][:, 0:h], rmax[b][:, h : 2 * h]
                    )
                    h //= 2
                # cross-partition max: ACT upcast -> PE transpose -> DVE reduce
                rm32 = small.tile([P, C], F32)
                nc.scalar.copy(out=rm32[:], in_=rmax[b][:, 0:C])
                mt = psum_m.tile([C, P], F32)
                nc.tensor.transpose(mt[:], rm32[:], identity[:])
                nc.vector.reduce_max(
                    out=xaug[b][0:C, 0:1], in_=mt[:], axis=mybir.AxisListType.X
                )
                # mean: fold the 8 q-residue groups, scale 1/HW via K=1 matmul
                srow = small.tile([1, C], F32)
                nc.vector.reduce_sum(
                    out=srow[:],
                    in_=ps[:].rearrange("p (r c) -> p c r", c=C),
                    axis=mybir.AxisListType.X,
                )
                av = psum_m.tile([C, 1], F32)
                nc.tensor.matmul(av[:], srow[:], one_hw[:], start=True, stop=True)
                nc.scalar.copy(out=xaug[b][0:C, 1:2], in_=av[:])

                # 7 residual PReLU blocks on (C, 2); bias via the augmented row
                for k in range(CONV_NUM):
                    y = psum_m.tile([C, 2], F32)
                    nc.tensor.matmul(
                        y[:], w_sb[:, k, :], xaug[b][:], start=True, stop=True
                    )
                    p_t = small.tile([C, 2], F32)
                    nc.vector.scalar_tensor_tensor(
                        p_t[:], y[:], 0.0, xaug[b][0:C, :], op0=AL.max, op1=AL.add
                    )
                    n_t = small.tile([C, 2], F32)
                    nc.vector.scalar_tensor_tensor(
                        n_t[:], y[:], 0.0,
                        a_sb[:, k : k + 1].to_broadcast([C, 2]),
                        op0=AL.min, op1=AL.mult,
                    )
                    nc.vector.tensor_add(xaug[b][0:C, :], p_t[:], n_t[:])

                # scores = sigmoid(max + avg); broadcast to partitions via PE
                ssum = small.tile([C, 1], F32)
                nc.vector.tensor_add(ssum[:], xaug[b][0:C, 0:1], xaug[b][0:C, 1:2])
                sc = small.tile([C, 1], F32)
                nc.scalar.activation(
                    out=sc[:], in_=ssum[:], func=mybir.ActivationFunctionType.Sigmoid
                )
                sc_t = psum_m.tile([1, C], F32)
                nc.tensor.transpose(sc_t[:], sc[:], identity[:C, :C])
                sc_sb = small.tile([1, C], F32)
                nc.scalar.copy(out=sc_sb[:], in_=sc_t[:])
                bc_ps = psum_m.tile([P, C], F32)
                nc.tensor.matmul(bc_ps[:], ones_row[:], sc_sb[:], start=True, stop=True)
                nc.scalar.copy(out=bcb[b][:], in_=bc_ps[:])

                # scale store-chunks in place (fp16 2x), ACT-cast each to an
                # fp32 staging buffer, and store via HWDGE (sync) -- the
                # SWDGE ring's slow engine 15 (descriptor-ring port
                # contention) otherwise straggles ~30us at the kernel tail.
                # The NEXT batch's running-max ops slot into this multiply
                # stream (data-paced by its loads) so its score chain can
                # start the moment its last load lands.
                bcv = bcb[b][:].unsqueeze(1).to_broadcast([P, QS, C])
                for j in range(NST):
                    sl = cb[b][:, j * QS : (j + 1) * QS, :]
                    nc.vector.tensor_mul(sl, sl, bcv)
                    if b + 1 < BPC and j in (1, 3, 5, 7):
                        emit_tmax(b + 1, j // 2)
            for j in range(NST):
                sl = cb[b][:, j * QS : (j + 1) * QS, :]
                stage = stpool.tile([P, QS, C], F32, tag="stage", name="stage")
                nc.scalar.copy(out=stage[:], in_=sl)
                nc.sync.dma_start(
                    out=out_r[b, :, j * QS : (j + 1) * QS, :], in_=stage[:]
                )

    _split_dma_waits(nc)
    return nc


def _prep_inputs(features, W1, b1, a1):
    feats = np.ascontiguousarray(features, dtype=np.float32).reshape(B, HW, C)
    W1 = np.asarray(W1, np.float32)
    b1 = np.asarray(b1, np.float32)
    # lhsT layout: waugT[c_in, k, c_out] = W1[k, c_out, c_in]; row C = b1[k, c_out]
    waugT = np.empty((C + 1, CONV_NUM, C), np.float32)
    waugT[:C] = np.transpose(W1, (2, 0, 1))
    waugT[C] = b1
    aT = np.ascontiguousarray(
        np.broadcast_to(np.asarray(a1, np.float32), (C, CONV_NUM))
    )
    ident = np.eye(P, dtype=np.float32)
    return feats, np.ascontiguousarray(waugT), aT, ident


def kernel(features, W1, b1, a1):
    global LAST_EXEC_NS
    feats, waugT, aT, ident = _prep_inputs(features, W1, b1, a1)
    nc = _build_nc()
    in_maps = [
        {
            "features": feats[i * BPC : (i + 1) * BPC],
            "waugT": waugT,
            "aT": aT,
            "ident": ident,
        }
        for i in range(NCORES)
    ]
    import os

    res = run_bass_kernel_spmd(
        nc,
        in_maps,
        list(range(NCORES)),
        trace=PROFILE,
        tmpdir=os.environ.get("BASS_TMPDIR"),
    )
    global LAST_RESULTS
    LAST_RESULTS = res
    LAST_EXEC_NS = res.exec_time_ns
    out = np.concatenate(
        [res.results[i]["out"].reshape(BPC, H, W, C) for i in range(NCORES)], axis=0
    )
    return out


# revision 13
# speedup vs baseline: 1.0947x; 1.0947x over previous
"""Trainium2 Bass kernel for ChannelFeatures (channel-attention style module).

Computes, per batch element b:
    x_max[b] = max over (H,W) of features[b]          # (C,)
    x_avg[b] = mean over (H,W) of features[b]         # (C,)
    7 residual blocks (shared weights on both branches):
        x = prelu(W1[k] @ x + b1[k], a1[k]) + x
    scores[b] = sigmoid(x_max[b] + x_avg[b])          # (C,)
    out[b] = features[b] * scores[b]                  # broadcast over (H,W)

Sharding: pure data parallel over batch — 16 batch elements across 8 cores,
2 per core, weights replicated. No cross-core communication.

Device strategy per core (2 batch elements, each (65536, 64) fp32 = 16.8 MB):
  The whole shard is cached in SBUF as fp16 (2 x 8 MB), so features are read
  from HBM exactly once and written exactly once (67.1 MB/core total traffic,
  the memory-roofline minimum). The fp32->fp16 cast happens inside the load
  DMA (SWDGE cast), the fp16->fp32 cast inside the store DMA.

  Layout: partition p holds spatial rows [p*512, (p+1)*512) of a batch
  (one contiguous 128 KB DRAM chunk per partition -> near-optimal DMA
  descriptors). Loads/stores move 4 MB chunks (128 q-rows each).

  Reductions: per-channel max via running elementwise fp16 tensor_max over
  chunks (DVE 16-bit 2x mode) + a log2 fold tree + PE transpose for the
  cross-partition step; per-channel sum via PE ones-matmul over fp16
  segments accumulated in PSUM (mean scale folded into a K=1 matmul).

  Recurrence: bias row folded into an augmented (65, C) weight matmul;
  PReLU+residual fused into 3 DVE scalar_tensor_tensor ops per block.

  Scores are broadcast to 128 partitions with a K=1 ones-matmul (no DRAM
  bounce), then each chunk is scaled in place (fp16 2x tensor_mul) and
  cast-stored back to fp32 DRAM.
"""

import numpy as np
from contextlib import ExitStack

import concourse.bass as bass
import concourse.tile as tile
from concourse import mybir
from concourse.bass_utils import run_bass_kernel_spmd

# Problem shapes (hardcoded per contract)
B, H, W, C = 16, 256, 256, 64
CONV_NUM = 7
NCORES = 8
BPC = B // NCORES          # batch elements per core
HW = H * W                 # 65536 spatial positions
P = 128                    # SBUF partitions
Q = HW // P                # 512 spatial rows per partition per batch
NLD = 2                    # load DMAs per batch (8.4 MB each)
QL = Q // NLD              # 256 q-rows per load chunk
NMX = 4                    # tensor_max granularity: quarter-batch views
CHF = (Q // NMX) * C       # 8192 elems per partition per max-op
QS = 64                    # q-rows per store chunk (2 MB fp32)
NST = Q // QS              # 8 store chunks per batch
FREE = Q * C               # 32768 fp16 elems per partition per batch
SEG = 512                  # PSUM-bank-width matmul segments
NSEG = FREE // SEG         # 64 sum-matmul segments per batch
F32 = mybir.dt.float32
F16 = mybir.dt.float16

AL = mybir.AluOpType

# test.py hooks: set PROFILE=True before calling kernel() to capture an NTFF
# trace; LAST_EXEC_NS then holds the max per-core HW execution time.
PROFILE = False
LAST_EXEC_NS = None
LAST_RESULTS = None


def _split_dma_waits(nc: bass.Bass) -> None:
    """The pinned walrus build rejects DMA instructions carrying more than one
    sync-wait ("Too many sync wait commands"). Tile's sem assignment is not
    transitively minimal, so slot-reuse instructions can get two waits
    (consumer release + WAW with the previous writer). Hoist all but the last
    wait onto wait-only EventSemaphore instructions on the same engine right
    before the instruction."""
    n = 0
    # num=200: outside every id Tile allocated (its end-of-kernel range-clear
    # covers the allocated block), so no collision with released Tile sems.
    dummy = nc.alloc_semaphore(name="wsplit_dummy", num=200)
    for fn in nc.m.functions:
        for blk in fn.blocks:
            new_insts = []
            for inst in blk.instructions:
                si = getattr(inst, "sync_info", None)
                if si is not None and len(si.on_wait) > 1:
                    for w in si.on_wait[:-1]:
                        ev = mybir.InstEventSemaphore(
                            name=f"WSPLIT-{n}", ins=[], outs=[]
                        )
                        n += 1
                        ev.engine = inst.engine
                        # Tick a dedicated dummy sem nobody waits on, so the
                        # simulator/race tooling (which require every
                        # instruction to carry an update) accept the carrier.
                        upd = mybir.SyncUpdate(
                            sync_type="semaphore",
                            id=dummy.num,
                            ant_name=dummy.name,
                            update_mode="sem-add-imm",
                            update_value=1,
                        )
                        ev.sync_info = mybir.SyncInfo(on_wait=[w], on_update=[upd])
                        new_insts.append(ev)
                    si.on_wait = [si.on_wait[-1]]
                new_insts.append(inst)
            blk.instructions = new_insts


def _build_nc() -> bass.Bass:
    nc = bass.Bass()
    feat = nc.declare_dram_parameter("features", [BPC, HW, C], F32, isOutput=False)
    # augmented lhsT: rows 0..C-1 = W1[k].T columns, row C = b1[k] (bias fold)
    waugT = nc.declare_dram_parameter("waugT", [C + 1, CONV_NUM, C], F32, isOutput=False)
    aT = nc.declare_dram_parameter("aT", [C, CONV_NUM], F32, isOutput=False)
    ident = nc.declare_dram_parameter("ident", [P, P], F32, isOutput=False)
    out = nc.declare_dram_parameter("out", [BPC, HW, C], F32, isOutput=True)

    feat_r = feat[:].rearrange("b (p q) c -> b p q c", p=P)
    out_r = out[:].rearrange("b (p q) c -> b p q c", p=P)

    with ExitStack() as ctx:
        tc = ctx.enter_context(tile.TileContext(nc))
        singles = ctx.enter_context(tc.tile_pool(name="singles", bufs=1))
        small = ctx.enter_context(tc.tile_pool(name="small", bufs=3))
        stpool = ctx.enter_context(tc.tile_pool(name="stage", bufs=3))
        psum_sum = ctx.enter_context(tc.tile_pool(name="psum_sum", bufs=1, space="PSUM"))
        psum_m = ctx.enter_context(tc.tile_pool(name="psum_m", bufs=1, space="PSUM"))

        # ---- constants (HWDGE loads; POOL queue stays clear for the stream)
        w_sb = singles.tile([C + 1, CONV_NUM, C], F32)   # [c_in_aug, k, c_out]
        nc.sync.dma_start(out=w_sb[:], in_=waugT[:])
        a_sb = singles.tile([C, CONV_NUM], F32)          # [c, k] (a1[k] per row)
        nc.sync.dma_start(out=a_sb[:], in_=aT[:])
        ones_mm = singles.tile([P, 1], F16)              # sum-matmul lhsT
        nc.vector.memset(ones_mm[:], 1.0)
        ones_row = singles.tile([1, P], F32)             # broadcast-matmul lhsT
        nc.vector.memset(ones_row[:], 1.0)
        one_hw = singles.tile([1, 1], F32)
        nc.vector.memset(one_hw[:], 1.0 / HW)
        identity = singles.tile([P, P], F32)
        nc.sync.dma_start(out=identity[:], in_=ident[:])

        # persistent per-batch state
        cb = [singles.tile([P, Q, C], F16, tag=f"cb{b}", name=f"cb{b}") for b in range(BPC)]
        cbf = [t[:].rearrange("p q c -> p (q c)") for t in cb]
        rmax_t = singles.tile([P, CHF], F16, tag="rmax", name="rmax")
        rmax = [rmax_t for _ in range(BPC)]
        xaug = [singles.tile([C + 1, 2], F32, tag=f"xaug{b}", name=f"xaug{b}") for b in range(BPC)]
        bcb = [singles.tile([P, C], F16, tag=f"bcb{b}", name=f"bcb{b}") for b in range(BPC)]

        for b in range(BPC):
            nc.vector.memset(xaug[b][C : C + 1, :], 1.0)

        # ---- loads: all chunk cast-DMAs up front on the gpsimd (SWDGE) ring
        for b in range(BPC):
            for j in range(NLD):
                nc.gpsimd.dma_start(
                    out=cb[b][:, j * QL : (j + 1) * QL, :],
                    in_=feat_r[b, :, j * QL : (j + 1) * QL, :],
                )

        def emit_tmax(b, j):
            # running elementwise max over quarter-batch views (fp16 2x TT);
            # first quarter initializes via copy (4x mode), no memset needed
            chunk = cbf[b][:, j * CHF : (j + 1) * CHF]
            if j == 0:
                nc.vector.tensor_copy(rmax[b][:], chunk)
            else:
                nc.vector.tensor_max(rmax[b][:], rmax[b][:], chunk)

        # ---- per-batch: reduce -> recurrence -> scores -> scale -> store
        for b in range(BPC):
            if b > 0:
                # scheduler-only fence: keep batch b's section behind batch
                # b-1's score chain in every engine's static order (the
                # scheduler's DMA cost model diverges from reality, which
                # otherwise buries b-1's tiny critical chain behind b's bulk)
                tc.no_sync_barrier()
            else:
                for j in range(NMX):
                    emit_tmax(b, j)
            # per-channel sums on PE: ones^T @ segment, PSUM-accumulated
            ps = psum_sum.tile([1, SEG], F32, tag=f"ps{b}")
            for s in range(NSEG):
                nc.tensor.matmul(
                    ps[:],
                    ones_mm[:],
                    cbf[b][:, s * SEG : (s + 1) * SEG],
                    start=(s == 0),
                    stop=(s == NSEG - 1),
                )
            # The fold->scores->multiply chain gates when stores can start;
            # pull it forward in the static schedule so batch 0's stores
            # overlap batch 1's loads on the DMA ring.
            with tc.high_priority(offset=100000):
                # fold tree: (P, 8192) -> (P, 64), c-alignment preserved
                h = CHF // 2
                while h >= C:
                    nc.vector.tensor_max(
                        rmax[b][:, 0:h], rmax[b][:, 0:h], rmax[b][:, h : 2 * h]
                    )
                    h //= 2
                # cross-partition max: ACT upcast -> PE transpose -> DVE reduce
                rm32 = small.tile([P, C], F32)
                nc.scalar.copy(out=rm32[:], in_=rmax[b][:, 0:C])
                mt = psum_m.tile([C, P], F32)
                nc.tensor.transpose(mt[:], rm32[:], identity[:])
                nc.vector.reduce_max(
                    out=xaug[b][0:C, 0:1], in_=mt[:], axis=mybir.AxisListType.X
                )
                # mean: fold the 8 q-residue groups, scale 1/HW via K=1 matmul
                srow = small.tile([1, C], F32)
                nc.vector.reduce_sum(
                    out=srow[:],
                    in_=ps[:].rearrange("p (r c) -> p c r", c=C),
                    axis=mybir.AxisListType.X,
                )
                av = psum_m.tile([C, 1], F32)
                nc.tensor.matmul(av[:], srow[:], one_hw[:], start=True, stop=True)
                nc.scalar.copy(out=xaug[b][0:C, 1:2], in_=av[:])

                # 7 residual PReLU blocks on (C, 2); bias via the augmented row
                for k in range(CONV_NUM):
                    y = psum_m.tile([C, 2], F32)
                    nc.tensor.matmul(
                        y[:], w_sb[:, k, :], xaug[b][:], start=True, stop=True
                    )
                    p_t = small.tile([C, 2], F32)
                    nc.vector.scalar_tensor_tensor(
                        p_t[:], y[:], 0.0, xaug[b][0:C, :], op0=AL.max, op1=AL.add
                    )
                    n_t = small.tile([C, 2], F32)
                    nc.vector.scalar_tensor_tensor(
                        n_t[:], y[:], 0.0,
                        a_sb[:, k : k + 1].to_broadcast([C, 2]),
                        op0=AL.min, op1=AL.mult,
                    )
                    nc.vector.tensor_add(xaug[b][0:C, :], p_t[:], n_t[:])

                # scores = sigmoid(max + avg); broadcast to partitions via PE
                ssum = small.tile([C, 1], F32)
                nc.vector.tensor_add(ssum[:], xaug[b][0:C, 0:1], xaug[b][0:C, 1:2])
                sc = small.tile([C, 1], F32)
                nc.scalar.activation(
                    out=sc[:], in_=ssum[:], func=mybir.ActivationFunctionType.Sigmoid
                )
                sc_t = psum_m.tile([1, C], F32)
                nc.tensor.transpose(sc_t[:], sc[:], identity[:C, :C])
                sc_sb = small.tile([1, C], F32)
                nc.scalar.copy(out=sc_sb[:], in_=sc_t[:])
                bc_ps = psum_m.tile([P, C], F32)
                nc.tensor.matmul(bc_ps[:], ones_row[:], sc_sb[:], start=True, stop=True)
                nc.scalar.copy(out=bcb[b][:], in_=bc_ps[:])

            # fence: without it the static scheduler hoists the next batch's
            # data-gated tensor_max ops ahead of this batch's tiny recurrence
            # STTs on the in-order DVE stream, stalling the score chain ~25us
            tc.no_sync_barrier()
            with tc.high_priority(offset=100000):
                # scale store-chunks in place (fp16 2x), ACT-cast each to an
                # fp32 staging buffer, and store via HWDGE (sync) -- the
                # SWDGE ring's slow engine 15 (descriptor-ring port
                # contention) otherwise straggles ~30us at the kernel tail.
                # The NEXT batch's running-max ops slot into this multiply
                # stream (data-paced by its loads) so its score chain can
                # start the moment its last load lands.
                bcv = bcb[b][:].unsqueeze(1).to_broadcast([P, QS, C])
                for j in range(NST):
                    sl = cb[b][:, j * QS : (j + 1) * QS, :]
                    nc.vector.tensor_mul(sl, sl, bcv)
                    if b + 1 < BPC and j in (1, 3, 5, 7):
                        emit_tmax(b + 1, j // 2)
            for j in range(NST):
                sl = cb[b][:, j * QS : (j + 1) * QS, :]
                stage = stpool.tile([P, QS, C], F32, tag="stage", name="stage")
                nc.scalar.copy(out=stage[:], in_=sl)
                nc.sync.dma_start(
                    out=out_r[b, :, j * QS : (j + 1) * QS, :], in_=stage[:]
                )

    _split_dma_waits(nc)
    return nc


def _prep_inputs(features, W1, b1, a1):
    feats = np.ascontiguousarray(features, dtype=np.float32).reshape(B, HW, C)
    W1 = np.asarray(W1, np.float32)
    b1 = np.asarray(b1, np.float32)
    # lhsT layout: waugT[c_in, k, c_out] = W1[k, c_out, c_in]; row C = b1[k, c_out]
    waugT = np.empty((C + 1, CONV_NUM, C), np.float32)
    waugT[:C] = np.transpose(W1, (2, 0, 1))
    waugT[C] = b1
    aT = np.ascontiguousarray(
        np.broadcast_to(np.asarray(a1, np.float32), (C, CONV_NUM))
    )
    ident = np.eye(P, dtype=np.float32)
    return feats, np.ascontiguousarray(waugT), aT, ident


def kernel(features, W1, b1, a1):
    global LAST_EXEC_NS
    feats, waugT, aT, ident = _prep_inputs(features, W1, b1, a1)
    nc = _build_nc()
    in_maps = [
        {
            "features": feats[i * BPC : (i + 1) * BPC],
            "waugT": waugT,
            "aT": aT,
            "ident": ident,
        }
        for i in range(NCORES)
    ]
    import os

    res = run_bass_kernel_spmd(
        nc,
        in_maps,
        list(range(NCORES)),
        trace=PROFILE,
        tmpdir=os.environ.get("BASS_TMPDIR"),
    )
    global LAST_RESULTS
    LAST_RESULTS = res
    LAST_EXEC_NS = res.exec_time_ns
    out = np.concatenate(
        [res.results[i]["out"].reshape(BPC, H, W, C) for i in range(NCORES)], axis=0
    )
    return out
